# revision 26
# baseline (speedup 1.0000x reference)
"""Bass/Trainium2 kernel for nn_MultiHeadAttention_76244259438975.

MHA forward: B=2, L=2048, D_MODEL=1024, H=16, Dk=Dv=64.
Returns (out [B,L,D], attn [B,H,L,L]).

Sharding: 8 cores = (batch 2) x (head-group 4); each core handles one batch
and 4 heads. Per core:
  A: PE-transpose x -> xT slices; project QT/KT/VT [d,2048] (f32r, DVE bias
     eviction); VT -> V [l,d] via PE transpose.  q,k first; v emitted after
     the S loops so its PE/DMA work overlaps the ACT-heavy exp phase.
  S (per head): S = QT_h.T @ KT_h -> exp (ACT scale=1/8, accum row sums,
     FD=1024) -> DVE per-partition normalize -> DMA attn out.  Recip row
     stored per qi-block, then DMA-transposed to DRAM for the AV phase.
  T (per head): S^T -> exp (f32r, unnormalized) -> AV matmul accumulates
     head_outT [64,2048] in PSUM over ki; normalized against the recip row
     (broadcast-DMA'd back) into the hoT slab (f32r).
  C: out_partial = hoT.T @ Wo_rows -> DRAM; host sums partials + bo.
"""

import numpy as np

import concourse.bass as bass
import concourse.tile as tile
from concourse import bacc, mybir
from concourse.bass_utils import run_bass_kernel_spmd
from concourse.masks import make_identity

F32 = mybir.dt.float32
F32R = mybir.dt.float32r
EXP = mybir.ActivationFunctionType.Exp

B, L, DM = 2, 2048, 1024
H, DK = 16, 64
HPC = 4            # heads per core
DHC = HPC * DK     # 256 head-dims per core
NCORES = 8

LB = L // 128      # 16 l-blocks
CS = DM // 128     # 8 c-subtiles
QT_SUB = DHC // 128  # 2


def build_program(reps: int = 1):
    nc = bacc.Bacc("TRN2", target_bir_lowering=False, debug=False)
    d = {}
    d["x_q"] = nc.dram_tensor("x_q", [128, CS, L], F32R, kind="ExternalInput").ap()
    d["x_k"] = nc.dram_tensor("x_k", [128, CS, L], F32R, kind="ExternalInput").ap()
    d["x_v"] = nc.dram_tensor("x_v", [128, CS, L], F32R, kind="ExternalInput").ap()
    d["wq"] = nc.dram_tensor("wq", [128, CS, DHC], F32R, kind="ExternalInput").ap()
    d["wk"] = nc.dram_tensor("wk", [128, CS, DHC], F32R, kind="ExternalInput").ap()
    d["wv"] = nc.dram_tensor("wv", [128, CS, DHC], F32R, kind="ExternalInput").ap()
    d["bq"] = nc.dram_tensor("bq", [128, QT_SUB], F32, kind="ExternalInput").ap()
    d["bk"] = nc.dram_tensor("bk", [128, QT_SUB], F32, kind="ExternalInput").ap()
    d["bv_row"] = nc.dram_tensor("bv_row", [1, DHC], F32, kind="ExternalInput").ap()
    d["wo"] = nc.dram_tensor("wo", [128, QT_SUB, DM], F32R, kind="ExternalInput").ap()
    d["identr"] = nc.dram_tensor("identr", [128, 128], F32R, kind="ExternalInput").ap()
    attn_out = nc.dram_tensor("attn_out", [HPC, L, L], F32, kind="ExternalOutput").ap()
    out_partial = nc.dram_tensor("out_partial", [L, DM], F32, kind="ExternalOutput").ap()
    recrow_dram = nc.dram_tensor("recrow_scratch", [HPC, L], F32).ap()

    with tile.TileContext(nc) as tc:
        with (
            tc.tile_pool(name="persist", bufs=1) as persist,
            tc.tile_pool(name="work", bufs=2) as work,
            tc.tile_pool(name="small", bufs=8) as small,
        ):
            ident = persist.tile([128, 128], F32R, name="ident")
            nc.sync.dma_start(ident[:], d["identr"][:])
            wo_sb = persist.tile([128, QT_SUB, DM], F32R, name="wo_sb")

            # persistent f32r slabs; VT shares storage with hoT (disjoint life)
            QT = persist.tile([128, QT_SUB, L], F32R, name="QT")
            KT = persist.tile([128, QT_SUB, L], F32R, name="KT")
            V = persist.tile([128, LB, DHC], F32R, name="V")
            vt_ho = persist.tile([128, QT_SUB, L], F32R, name="vt_ho")
            VT = vt_ho
            hoT = vt_ho

            def phase_a_input(phA, psA, name, wkey, bsrc, dst, after_lt=None):
                w_sb = phA.tile([128, CS, DHC], F32R, tag="w_sb")
                nc.sync.dma_start(w_sb[:], d[wkey][:])
                b_sb = phA.tile([128, QT_SUB], F32, tag="b_sb")
                nc.sync.dma_start(b_sb[:], bsrc)
                for lt in range(L // 512):
                    xTs = phA.tile([128, CS, 512], F32R, tag="xTs", bufs=3)
                    nc.sync.dma_start(
                        xTs[:], d["x_" + name][:, :, lt * 512:(lt + 1) * 512])
                    for mg in range(QT_SUB):
                        pp = psA.tile([128, 512], F32, tag="pp", bufs=2)
                        for cs in range(CS):
                            nc.tensor.matmul(
                                pp[:], w_sb[:, cs, mg * 128:(mg + 1) * 128],
                                xTs[:, cs, :], start=(cs == 0), stop=(cs == CS - 1))
                        nc.vector.tensor_scalar_add(
                            dst[:, mg, lt * 512:(lt + 1) * 512], pp[:],
                            b_sb[:, mg:mg + 1])
                    if after_lt is not None:
                        after_lt(lt)

            for _rep in range(reps):
                with tc.tile_pool(name="psX", bufs=2, space="PSUM") as psX:

                    def s_rows(h, qo_list):
                        bp = 64 * (h % 2)
                        sub = h // 2
                        QTh = QT[bp:bp + 64, sub]
                        KTh = KT[bp:bp + 64, sub]
                        rec_slab = rec_slabs[h]
                        for qo in qo_list:
                            attn_row = work.tile([128, L], F32,
                                                 tag="attn_row", bufs=3)
                            acc2 = small.tile([128, 2], F32, tag="acc2")
                            for hf in range(2):
                                ps = psX.tile([128, 1024], F32, tag="ps", bufs=2)
                                for kq in range(2):
                                    kt = hf * 2 + kq
                                    nc.tensor.matmul(
                                        ps[:, kq * 512:(kq + 1) * 512],
                                        QTh[:, qo * 128:(qo + 1) * 128],
                                        KTh[:, kt * 512:(kt + 1) * 512],
                                        start=True, stop=True)
                                nc.scalar.activation(
                                    attn_row[:, hf * 1024:(hf + 1) * 1024],
                                    ps[:], EXP, scale=0.125,
                                    accum_out=acc2[:, hf:hf + 1])
                            rsum = small.tile([128, 1], F32, tag="rsum")
                            nc.vector.reduce_sum(rsum[:], acc2[:],
                                                 axis=mybir.AxisListType.X)
                            nc.vector.reciprocal(rec_slab[:, qo:qo + 1], rsum[:])
                            nc.vector.tensor_scalar_mul(
                                attn_row[:], attn_row[:], rec_slab[:, qo:qo + 1])
                            nc.sync.dma_start(
                                attn_out[h, qo * 128:(qo + 1) * 128, :],
                                attn_row[:])
                            if qo == LB - 1:
                                nc.sync.dma_start(
                                    recrow_dram[h].rearrange("(b p) -> p b",
                                                             p=128),
                                    rec_slab[:])

                    def st_chunk(h, psAV, pav, ko_list):
                        bp = 64 * (h % 2)
                        sub = h // 2
                        QTh = QT[bp:bp + 64, sub]
                        KTh = KT[bp:bp + 64, sub]
                        for ko in ko_list:
                            expT = work.tile([128, L], F32R, tag="expT")
                            for hf in range(2):
                                ps2 = psX.tile([128, 1024], F32, tag="ps",
                                               bufs=2)
                                for kq in range(2):
                                    qt = hf * 2 + kq
                                    nc.tensor.matmul(
                                        ps2[:, kq * 512:(kq + 1) * 512],
                                        KTh[:, ko * 128:(ko + 1) * 128],
                                        QTh[:, qt * 512:(qt + 1) * 512],
                                        start=True, stop=True)
                                nc.scalar.activation(
                                    expT[:, hf * 1024:(hf + 1) * 1024],
                                    ps2[:], EXP, scale=0.125)
                            for qt in range(4):
                                nc.tensor.matmul(
                                    pav[:, qt * 512:(qt + 1) * 512],
                                    V[:, ko, h * 64:(h + 1) * 64],
                                    expT[:, qt * 512:(qt + 1) * 512],
                                    start=(ko == 0), stop=(ko == LB - 1))

                    def st_fin(h, pav):
                        bp = 64 * (h % 2)
                        sub = h // 2
                        recrow = work.tile([64, L], F32, tag="recrow")
                        nc.sync.dma_start(
                            recrow[:],
                            recrow_dram[h].unsqueeze(0).to_broadcast([64, L]))
                        nc.vector.tensor_mul(hoT[bp:bp + 64, sub], pav[:],
                                             recrow[:])

                    def zip_st_s(hst, psAV, hs=None, s_qos=None):
                        # interleave ST(hst) per-ko with S(hs) per-row so the
                        # attn DMA stream stays fed at an even rate
                        pav = psAV.tile([64, L], F32, tag="pav", name="pav")
                        rows = [q for c in (s_qos or []) for q in c]
                        for ko in range(LB):
                            st_chunk(hst, psAV, pav, [ko])
                            if hs is not None and ko < len(rows):
                                s_rows(hs, [rows[ko]])
                        if hs is not None:
                            s_rows(hs, rows[LB:])
                        st_fin(hst, pav)

                    rec_slabs = [
                        small.tile([128, LB], F32, tag=f"rec{h}", bufs=1,
                                   name=f"rec{h}")
                        for h in range(HPC)]
                    with (
                        tc.tile_pool(name="phA", bufs=2) as phA,
                        tc.tile_pool(name="psA", bufs=2, space="PSUM") as psA,
                    ):
                        def vtv_lt(lt):
                            for lb in range(lt * 4, lt * 4 + 4):
                                for ds2 in range(QT_SUB):
                                    pt2 = psA.tile([128, 128], F32R, tag="pt2",
                                                   bufs=2, name="pt2")
                                    nc.tensor.transpose(
                                        pt2[:],
                                        VT[:, ds2, lb * 128:(lb + 1) * 128],
                                        ident[:])
                                    nc.vector.tensor_copy(
                                        V[:, lb, ds2 * 128:(ds2 + 1) * 128],
                                        pt2[:])

                        phase_a_input(phA, psA, "v", "wv",
                                      d["bv_row"][0].rearrange("(s p) -> p s",
                                                               p=128),
                                      VT, after_lt=vtv_lt)
                        phase_a_input(phA, psA, "k", "wk", d["bk"][:], KT)
                        phase_a_input(phA, psA, "q", "wq", d["bq"][:], QT)

                    with tc.tile_pool(name="psAV", bufs=1,
                                      space="PSUM") as psAV:
                        full = [list(range(c * 4, c * 4 + 4))
                                for c in range(4)]
                        for h in range(HPC):
                            zip_st_s(h, psAV, h, full)

                    nc.sync.dma_start(wo_sb[:], d["wo"][:])
                    with tc.tile_pool(name="psC", bufs=4, space="PSUM") as psC:
                        for lb in range(LB):
                            o_sb = work.tile([128, DM], F32, tag="o_sb",
                                             bufs=3)
                            for et in range(2):
                                po = psC.tile([128, 512], F32, tag="po", bufs=4)
                                for s2 in range(QT_SUB):
                                    nc.tensor.matmul(
                                        po[:],
                                        hoT[:, s2, lb * 128:(lb + 1) * 128],
                                        wo_sb[:, s2, et * 512:(et + 1) * 512],
                                        start=(s2 == 0),
                                        stop=(s2 == QT_SUB - 1))
                                nc.vector.tensor_copy(
                                    o_sb[:, et * 512:(et + 1) * 512], po[:])
                                nc.sync.dma_start(
                                    out_partial[lb * 128:(lb + 1) * 128,
                                                et * 512:(et + 1) * 512],
                                    o_sb[:, et * 512:(et + 1) * 512])
    nc.compile()
    return nc


_NC_CACHE = {}


def _get_nc(reps=1):
    if reps not in _NC_CACHE:
        _NC_CACHE[reps] = build_program(reps)
    return _NC_CACHE[reps]


def make_in_maps(q, k, v, Wq, bq, Wk, bk, Wv, bv, Wo, bo):
    q, k, v = (np.asarray(a, np.float32) for a in (q, k, v))
    Wq, bq, Wk, bk, Wv, bv, Wo, bo = (
        np.asarray(a, np.float32) for a in (Wq, bq, Wk, bk, Wv, bv, Wo, bo))
    in_maps = []
    for c in range(NCORES):
        b = c // 4
        hg = c % 4
        hs = slice(hg * DHC, (hg + 1) * DHC)
        m = {
            "x_q": np.ascontiguousarray(q[b].T.reshape(CS, 128, L).transpose(1, 0, 2)),
            "x_k": np.ascontiguousarray(k[b].T.reshape(CS, 128, L).transpose(1, 0, 2)),
            "x_v": np.ascontiguousarray(v[b].T.reshape(CS, 128, L).transpose(1, 0, 2)),
            "wq": np.ascontiguousarray(Wq[:, hs].reshape(CS, 128, DHC).transpose(1, 0, 2)),
            "wk": np.ascontiguousarray(Wk[:, hs].reshape(CS, 128, DHC).transpose(1, 0, 2)),
            "wv": np.ascontiguousarray(Wv[:, hs].reshape(CS, 128, DHC).transpose(1, 0, 2)),
            "bq": np.ascontiguousarray(bq[hs].reshape(QT_SUB, 128).T),
            "bk": np.ascontiguousarray(bk[hs].reshape(QT_SUB, 128).T),
            "bv_row": np.ascontiguousarray(bv[hs].reshape(1, DHC)),
            "wo": np.ascontiguousarray(Wo[hs].reshape(QT_SUB, 128, DM).transpose(1, 0, 2)),
            "identr": np.eye(128, dtype=np.float32),
        }
        in_maps.append(m)
    return in_maps


def assemble(results, bo):
    out = np.zeros((B, L, DM), np.float32)
    attn = np.empty((B, H, L, L), np.float32)
    for c in range(NCORES):
        b = c // 4
        hg = c % 4
        attn[b, hg * HPC:(hg + 1) * HPC] = results[c]["attn_out"]
        out[b] += results[c]["out_partial"]
    out += np.asarray(bo, np.float32)[None, None, :]
    return out, attn


def kernel(q, k, v, Wq, bq, Wk, bk, Wv, bv, Wo, bo):
    nc = _get_nc(1)
    in_maps = make_in_maps(q, k, v, Wq, bq, Wk, bk, Wv, bv, Wo, bo)
    res = run_bass_kernel_spmd(nc, in_maps, core_ids=list(range(NCORES)))
    return assemble(res.results, bo)
